# revision 18
# baseline (speedup 1.0000x reference)
"""Trainium2 Bass kernel for nn_DiffusionModel_56822417326086.

Causal multi-head self-attention block:
    qkv = x @ w_qkv ; split into 8 heads of 64
    e = (q @ k^T) * DH^-0.5 ; causal + key-padding mask ; a = softmax(e)
    o = a @ v ; y = o @ w_out + b_out ; y *= m

Sharding (8 cores, zero collectives):
    core c -> batch b = c // 2, head-quad q = c % 2 (heads 4q..4q+3).
    Each core computes q/k/v for its 4 heads over its whole batch, full
    causal attention for those heads, and the partial output projection
    y_partial = o[heads] @ w_out[head rows].  Host sums the two partials
    per batch (linear unshard), adds b_out, applies the query-side mask.

On-device layout notes:
  - scores are computed TRANSPOSED: sT[key, query] so that the A@V
    contraction (over keys) has keys on the partition dim.
  - softmax denominators come for free as a 65th "ones" column of V.
  - no max-subtraction in softmax: scores are O(1) here, exp is safe.
  - matmuls run as float32r (fp32 data on the fast PE path).
"""

import numpy as np
from contextlib import ExitStack

B, T, D, H = 4, 2048, 512, 8
DH = D // H
SCALE = DH ** -0.5
NEG = -1.0e30
QC = 256           # query-chunk (free dim of score matmuls)
NQC = T // QC      # 8
KB = 128           # key-block (partition dim of score tiles)

_CACHE = {}


def _build_program(stage=5):
    import concourse.mybir as mybir
    import concourse.tile as tile
    from concourse import bacc

    f32 = mybir.dt.float32
    f32r = mybir.dt.float32r
    Exp = mybir.ActivationFunctionType.Exp

    nc = bacc.Bacc("TRN2", target_bir_lowering=False, debug=False)

    xT_d = nc.dram_tensor("xT", [D, T], f32r, kind="ExternalInput").ap()
    wq_d = nc.dram_tensor("wq2", [2, D, 128], f32r, kind="ExternalInput").ap()
    wk_d = nc.dram_tensor("wk2", [2, D, 128], f32r, kind="ExternalInput").ap()
    wv_d = nc.dram_tensor("wv4", [D, 256], f32r, kind="ExternalInput").ap()
    wo_d = nc.dram_tensor("wo4", [256, D], f32r, kind="ExternalInput").ap()
    dm_d = nc.dram_tensor("dm2", [2, 128, 512], f32, kind="ExternalInput").ap()
    mk_d = nc.dram_tensor("mkey", [T, 1], f32, kind="ExternalInput").ap()
    y_d = nc.dram_tensor("y", [T, D], f32, kind="ExternalOutput").ap()

    with tile.TileContext(nc) as tc, ExitStack() as ctx:
        consts = ctx.enter_context(tc.tile_pool(name="consts", bufs=1))
        work = ctx.enter_context(tc.tile_pool(name="work", bufs=2))
        exp_pool = ctx.enter_context(tc.tile_pool(name="exp", bufs=3))
        ps_big = ctx.enter_context(tc.tile_pool(name="psb", bufs=3, space="PSUM"))
        ps_o = ctx.enter_context(tc.tile_pool(name="pso", bufs=2, space="PSUM"))
        ps_bc = ctx.enter_context(tc.tile_pool(name="psbc", bufs=1, space="PSUM"))

        # ---- constant loads -------------------------------------------------
        xT = consts.tile([128, 4, T], f32r)          # x[b].T, D on partitions
        for kc in range(4):
            nc.sync.dma_start(xT[:, kc, :], xT_d[kc * 128:(kc + 1) * 128, :])
        wq = consts.tile([128, 2, 4, 128], f32r)     # [pair][kchunk] -> (qA|qB)
        wk = consts.tile([128, 2, 4, 128], f32r)
        for p in range(2):
            for kc in range(4):
                nc.sync.dma_start(wq[:, p, kc, :], wq_d[p, kc * 128:(kc + 1) * 128, :])
                nc.sync.dma_start(wk[:, p, kc, :], wk_d[p, kc * 128:(kc + 1) * 128, :])
        wv = consts.tile([128, 4, 256], f32r)        # [kchunk] -> 4 heads x 64
        for kc in range(4):
            nc.sync.dma_start(wv[:, kc, :], wv_d[kc * 128:(kc + 1) * 128, :])
        wo = consts.tile([128, 2, D], f32r)          # [pair] w_out rows
        for p in range(2):
            nc.sync.dma_start(wo[:, p, :], wo_d[p * 128:(p + 1) * 128, :])
        dm = consts.tile([128, 2, 512], f32)        # diagonal masks (x2 tiled)
        for v_ in range(2):
            nc.sync.dma_start(dm[:, v_, :], dm_d[v_])
        mk = consts.tile([128, 16], f32)            # key-padding multiplier
        for rc in range(16):
            nc.sync.dma_start(mk[:, rc:rc + 1], mk_d[rc * 128:(rc + 1) * 128, :])

        # ---- qkv projection -------------------------------------------------
        # Per-head transposed q/k at partitions 0-63 (base-64 matmul operands
        # are broken on this runtime, so head B is DMA-shifted down).
        qTa = consts.tile([64, 2, T], f32r)
        qTb = consts.tile([64, 2, T], f32r)
        kTa = consts.tile([64, 2, T], f32r)
        kTb = consts.tile([64, 2, T], f32r)
        for p in range(2):
            for rc4 in range(4):
                sl = slice(rc4 * 512, (rc4 + 1) * 512)
                psq = ps_big.tile([128, 512], f32, tag="scores")
                psk = ps_big.tile([128, 512], f32, tag="scores")
                for kc in range(4):
                    nc.tensor.matmul(psq[:], wq[:, p, kc, :], xT[:, kc, sl],
                                     start=kc == 0, stop=kc == 3)
                    nc.tensor.matmul(psk[:], wk[:, p, kc, :], xT[:, kc, sl],
                                     start=kc == 0, stop=kc == 3)
                nc.vector.tensor_copy(qTa[:, p, sl], psq[0:64, :])
                nc.vector.tensor_copy(kTa[:, p, sl], psk[0:64, :])
                shq = work.tile([128, 512], f32r, tag="sumscr")
                nc.vector.tensor_copy(shq[64:128, :], psq[64:128, :])
                nc.sync.dma_start(qTb[:, p, sl], shq[64:128, :])
                shk = work.tile([128, 512], f32r, tag="sumscr")
                nc.vector.tensor_copy(shk[64:128, :], psk[64:128, :])
                nc.sync.dma_start(kTb[:, p, sl], shk[64:128, :])

        # v in normal layout + ones column, scaled by key mask:
        # vsb[:, rc, h, 0:64] = (x @ w_v)[rc rows, head h] * mk ; [..., 64] = mk
        vsb = consts.tile([128, 16, 4, 65], f32r)
        ones41 = consts.tile([128, 4, 1], f32)
        nc.vector.memset(ones41[:], 1.0)
        for rc in range(16):
            psv = ps_big.tile([128, 4, 64], f32, tag="scores")
            for kc in range(4):
                nc.tensor.matmul(psv[:], xT[:, kc, rc * 128:(rc + 1) * 128],
                                 wv[:, kc, :], start=kc == 0, stop=kc == 3)
            nc.vector.tensor_scalar_mul(vsb[:, rc, :, 0:64], psv[:], mk[:, rc:rc + 1])
            nc.vector.tensor_scalar_mul(vsb[:, rc, :, 64:65], ones41[:],
                                        mk[:, rc:rc + 1])

        if stage <= 1:
            yt1 = work.tile([128, 512], f32, tag="ysb")
            nc.vector.tensor_copy(yt1[0:64, :], qTa[:, 0, 0:512])
            nc.sync.dma_start(y_d[0:64, :], yt1[0:64, :])
            yt2 = work.tile([128, 512], f32, tag="ysb")
            nc.vector.tensor_copy(yt2[:, 0:256], vsb[:, 0, :, 0:64])
            nc.sync.dma_start(y_d[128:256, 0:256], yt2[:, 0:256])

        # ---- attention ------------------------------------------------------
        # Unnormalized head outputs (oUA/oUB: partitions 0-63) + denominators.
        oUA = consts.tile([64, 2, T], f32)
        oUB = consts.tile([64, 2, T], f32)
        sums_stage = consts.tile([16, 512], f32)    # [p*8+qc] -> (dA | dB)

        for p in range(2 if stage >= 2 else 0):
            for qc in range(NQC):
                nkb = 2 * (qc + 1)
                qsl = slice(qc * QC, (qc + 1) * QC)
                if stage >= 3:
                    oA = ps_o.tile([128, 256], f32, tag="oA")
                    oB = ps_o.tile([128, 256], f32, tag="oB")
                avq = []        # deferred A@V matmuls (software pipelining)
                for kb in range(nkb):
                    ksl = slice(kb * KB, (kb + 1) * KB)
                    sps = ps_big.tile([128, 512], f32, tag="scores")
                    nc.tensor.matmul(sps[:, 0:256], kTa[:, p, ksl],
                                     qTa[:, p, qsl], start=True, stop=True)
                    if stage >= 2.2:
                        nc.tensor.matmul(sps[:, 256:512], kTb[:, p, ksl],
                                         qTb[:, p, qsl], start=True, stop=True)
                    ex = None
                    if stage >= 2.25:
                        if kb >= nkb - 2:
                            nc.vector.tensor_add(sps[:], sps[:],
                                                 dm[:, kb - (nkb - 2), :])
                        ex = exp_pool.tile([128, 512], f32r, tag="exp")
                        nc.scalar.activation(ex[:], sps[:], Exp, scale=SCALE)
                    if stage < 3:
                        if p == 0 and qc == 0 and kb == 0:
                            yt = work.tile([128, 512], f32, tag="ysb")
                            nc.vector.tensor_copy(yt[:], ex[:] if ex is not None
                                                  else sps[:])
                            nc.sync.dma_start(y_d[0:128, :], yt[:])
                        continue
                    avq.append((kb, ex))
                    if len(avq) > 1:
                        _em(nc, avq.pop(0), oA, oB, vsb, p, nkb)
                if stage < 3:
                    continue
                _em(nc, avq.pop(0), oA, oB, vsb, p, nkb)

                # denominators (row 64) -> sums_stage[p*8+qc]
                scr = work.tile([128, 512], f32, tag="sumscr")
                nc.vector.tensor_copy(scr[64:65, 0:256], oA[64:65, :])
                nc.vector.tensor_copy(scr[64:65, 256:512], oB[64:65, :])
                idx = p * 8 + qc
                nc.sync.dma_start(sums_stage[idx:idx + 1, :], scr[64:65, :])
                # unnormalized outputs -> SBUF
                nc.vector.tensor_copy(oUA[:, p, qsl], oA[0:64, :])
                nc.vector.tensor_copy(oUB[:, p, qsl], oB[0:64, :])

        if stage == 3:
            yt = work.tile([128, 512], f32, tag="ysb")
            nc.vector.tensor_copy(yt[0:64, :], oUA[:, 0, 0:512])
            nc.sync.dma_start(y_d[0:64, :], yt[0:64, :])

        # ---- normalize ------------------------------------------------------
        recips_f = consts.tile([16, 512], f32)
        if stage >= 4:
            nc.vector.reciprocal(recips_f[:], sums_stage[:])
        recips = consts.tile([16, 512], f32r)
        if stage >= 4:
            nc.vector.tensor_copy(recips[:], recips_f[:])
        ones64 = consts.tile([1, 64], f32)
        nc.vector.memset(ones64[:], 1.0)
        ones64r = consts.tile([1, 64], f32r)
        nc.vector.tensor_copy(ones64r[:], ones64[:])

        # oTn2: [128, pair, T] packed for the output projection lhsT.
        oTn2 = consts.tile([128, 2, T], f32r)
        for p in range(2 if stage >= 4 else 0):
            for qc in range(NQC):
                idx = p * 8 + qc
                qsl = slice(qc * QC, (qc + 1) * QC)
                rec = work.tile([1, 512], f32r, tag="rec")
                nc.sync.dma_start(rec[:], recips[idx:idx + 1, :])
                bcA = ps_bc.tile([64, 256], f32, tag="bc")
                nc.tensor.matmul(bcA[:], ones64r[:], rec[0:1, 0:256],
                                 start=True, stop=True)
                nc.vector.tensor_mul(oTn2[0:64, p, qsl], oUA[:, p, qsl], bcA[:])
                bcB = ps_bc.tile([64, 256], f32, tag="bc")
                nc.tensor.matmul(bcB[:], ones64r[:], rec[0:1, 256:512],
                                 start=True, stop=True)
                scrB = work.tile([64, 256], f32r, tag="scrB")
                nc.vector.tensor_mul(scrB[:], oUB[:, p, qsl], bcB[:])
                # partition shift 0-63 -> 64-127 (DVE lanes are partition-locked)
                nc.sync.dma_start(oTn2[64:128, p, qsl], scrB[:])

        if stage == 4:
            yt = work.tile([128, 512], f32, tag="ysb")
            nc.vector.tensor_copy(yt[:], oTn2[:, 0, 0:512])
            nc.sync.dma_start(y_d[0:128, :], yt[:])

        # ---- output projection ---------------------------------------------
        for rc in range(16 if stage >= 5 else 0):
            rsl = slice(rc * 128, (rc + 1) * 128)
            psy = ps_big.tile([128, 512], f32, tag="scores")
            for p in range(2):
                nc.tensor.matmul(psy[:], oTn2[:, p, rsl], wo[:, p, :],
                                 start=p == 0, stop=p == 1)
            yt = work.tile([128, 512], f32, tag="ysb")
            nc.vector.tensor_copy(yt[:], psy[:])
            nc.sync.dma_start(y_d[rsl, :], yt[:])

    nc.compile()
    return nc


def _em(nc, item, oA, oB, vsb, p, nkb):
    """Emit the deferred A@V accumulation for one key block."""
    kb, ex = item
    nc.tensor.matmul(oA[0:65, :], vsb[:, kb, 2 * p, :], ex[:, 0:256],
                     start=kb == 0, stop=kb == nkb - 1)
    nc.tensor.matmul(oB[0:65, :], vsb[:, kb, 2 * p + 1, :], ex[:, 256:512],
                     start=kb == 0, stop=kb == nkb - 1)


def _diag_masks():
    i = np.arange(QC)[None, :]
    j = np.arange(KB)[:, None]
    m0 = np.where(i >= j, 0.0, NEG).astype(np.float32)        # even diag block
    m1 = np.where(i >= j + KB, 0.0, NEG).astype(np.float32)   # odd diag block
    return np.tile(m0, (1, 2)).copy(), np.tile(m1, (1, 2)).copy()


def _prep_inputs(x, m, w_qkv, w_out):
    """Per-core input maps for SPMD dispatch."""
    dm0, dm1 = _diag_masks()
    dm2 = np.stack([dm0, dm1]).astype(np.float32)
    wq_full = w_qkv[:, 0:D]
    wk_full = w_qkv[:, D:2 * D]
    wv_full = w_qkv[:, 2 * D:3 * D]
    in_maps = []
    for c in range(8):
        b, q = c // 2, c % 2
        hsl = slice(4 * q * DH, (4 * q + 4) * DH)
        wq2 = np.stack([
            np.concatenate([wq_full[:, (4 * q + 2 * p) * DH:(4 * q + 2 * p + 1) * DH],
                            wq_full[:, (4 * q + 2 * p + 1) * DH:(4 * q + 2 * p + 2) * DH]],
                           axis=1)
            for p in range(2)])
        wk2 = np.stack([
            np.concatenate([wk_full[:, (4 * q + 2 * p) * DH:(4 * q + 2 * p + 1) * DH],
                            wk_full[:, (4 * q + 2 * p + 1) * DH:(4 * q + 2 * p + 2) * DH]],
                           axis=1)
            for p in range(2)])
        in_maps.append({
            "xT": np.ascontiguousarray(x[b].T).astype(np.float32),
            "wq2": np.ascontiguousarray(wq2).astype(np.float32),
            "wk2": np.ascontiguousarray(wk2).astype(np.float32),
            "wv4": np.ascontiguousarray(wv_full[:, hsl]).astype(np.float32),
            "wo4": np.ascontiguousarray(w_out[hsl, :]).astype(np.float32),
            "dm2": dm2,
            "mkey": np.ascontiguousarray((m[b] != 0).astype(np.float32)[:, None]),
        })
    return in_maps


def _execute(inputs, trace=False, stage=5):
    from concourse.bass_utils import run_bass_kernel_spmd

    key = f"nc{stage}"
    if key not in _CACHE:
        _CACHE[key] = _build_program(stage)
    nc = _CACHE[key]

    x = np.asarray(inputs["x"], np.float32)
    m = np.asarray(inputs["m"], np.float32)
    w_qkv = np.asarray(inputs["w_qkv"], np.float32)
    w_out = np.asarray(inputs["w_out"], np.float32)
    b_out = np.asarray(inputs["b_out"], np.float32)

    in_maps = _prep_inputs(x, m, w_qkv, w_out)
    res = run_bass_kernel_spmd(nc, in_maps, core_ids=list(range(8)), trace=trace)

    y = np.empty((B, T, D), np.float32)
    for b in range(B):
        y[b] = res.results[2 * b]["y"] + res.results[2 * b + 1]["y"]
    y += b_out[None, None, :]
    y *= m[..., None]
    return y, res


def kernel(**inputs) -> np.ndarray:
    y, _ = _execute(inputs, trace=False)
    return y


# revision 20
# speedup vs baseline: 1.0925x; 1.0925x over previous
"""Trainium2 Bass kernel for nn_DiffusionModel_56822417326086.

Causal multi-head self-attention block:
    qkv = x @ w_qkv ; split into 8 heads of 64
    e = (q @ k^T) * DH^-0.5 ; causal + key-padding mask ; a = softmax(e)
    o = a @ v ; y = o @ w_out + b_out ; y *= m

Sharding (8 cores, zero collectives):
    core c -> batch b = c // 2, head-quad q = c % 2 (heads 4q..4q+3).
    Each core computes q/k/v for its 4 heads over its whole batch, full
    causal attention for those heads, and the partial output projection
    y_partial = o[heads] @ w_out[head rows].  Host sums the two partials
    per batch (linear unshard), adds b_out, applies the query-side mask.

On-device layout notes:
  - scores are computed TRANSPOSED: sT[key, query] so that the A@V
    contraction (over keys) has keys on the partition dim.
  - softmax denominators come for free as a 65th "ones" column of V.
  - no max-subtraction in softmax: scores are O(1) here, exp is safe.
  - matmuls run as float32r (fp32 data on the fast PE path).
  - all matmul operands live at partition base 0 (base-64 operands fault
    on this runtime), so q/k are stored per-head at partitions 0-63.
  - all 4 heads of one key block share a 2-bank PSUM tile [128, 1024]
    so one ACT Exp op covers them (ACT per-op overhead is ~250 ns).
"""

import numpy as np
from contextlib import ExitStack

B, T, D, H = 4, 2048, 512, 8
DH = D // H
SCALE = DH ** -0.5
NEG = -1.0e30
QC = 256           # query-chunk (free dim of score matmuls)
NQC = T // QC      # 8
KB = 128           # key-block (partition dim of score tiles)

_CACHE = {}


def _build_program():
    import concourse.mybir as mybir
    import concourse.tile as tile
    from concourse import bacc

    f32 = mybir.dt.float32
    f32r = mybir.dt.float32r
    Exp = mybir.ActivationFunctionType.Exp

    nc = bacc.Bacc("TRN2", target_bir_lowering=False, debug=False)

    xT_d = nc.dram_tensor("xT", [D, T], f32r, kind="ExternalInput").ap()
    wq_d = nc.dram_tensor("wq2", [2, D, 128], f32r, kind="ExternalInput").ap()
    wk_d = nc.dram_tensor("wk2", [2, D, 128], f32r, kind="ExternalInput").ap()
    wv_d = nc.dram_tensor("wv4", [D, 256], f32r, kind="ExternalInput").ap()
    wo_d = nc.dram_tensor("wo4", [256, D], f32r, kind="ExternalInput").ap()
    dm_d = nc.dram_tensor("dm4", [2, 128, 1024], f32, kind="ExternalInput").ap()
    mk_d = nc.dram_tensor("mkey", [T, 1], f32, kind="ExternalInput").ap()
    y_d = nc.dram_tensor("y", [T, D], f32, kind="ExternalOutput").ap()

    with tile.TileContext(nc) as tc, ExitStack() as ctx:
        consts = ctx.enter_context(tc.tile_pool(name="consts", bufs=1))
        work = ctx.enter_context(tc.tile_pool(name="work", bufs=2))
        ps_big = ctx.enter_context(tc.tile_pool(name="psb", bufs=2, space="PSUM"))
        ps_o = ctx.enter_context(tc.tile_pool(name="pso", bufs=1, space="PSUM"))

        # ---- persistent tiles ----------------------------------------------
        qTa = consts.tile([64, 2, T], f32r)   # [head-in-pair A][pair] q^T
        qTb = consts.tile([64, 2, T], f32r)
        kTa = consts.tile([64, 2, T], f32r)
        kTb = consts.tile([64, 2, T], f32r)
        vsb = consts.tile([128, 16, 4, 65], f32r)
        wo = consts.tile([128, 2, D], f32r)
        dm = consts.tile([128, 2, 1024], f32)
        mk = consts.tile([128, 16], f32)
        ones41 = consts.tile([128, 4, 1], f32)
        oUA = consts.tile([64, 2, T], f32)
        oUB = consts.tile([64, 2, T], f32)
        sums_stage = consts.tile([16, 512], f32)
        recips_f = consts.tile([16, 512], f32)
        recips = consts.tile([16, 512], f32r)
        ones64 = consts.tile([1, 64], f32)
        ones64r = consts.tile([1, 64], f32r)
        oTn2 = consts.tile([128, 2, T], f32r)

        nc.vector.memset(ones41[:], 1.0)
        nc.vector.memset(ones64[:], 1.0)
        nc.vector.tensor_copy(ones64r[:], ones64[:])
        for p in range(2):
            nc.sync.dma_start(wo[:, p, :], wo_d[p * 128:(p + 1) * 128, :])
        for v_ in range(2):
            nc.sync.dma_start(dm[:, v_, :], dm_d[v_])
        for rc in range(16):
            nc.sync.dma_start(mk[:, rc:rc + 1], mk_d[rc * 128:(rc + 1) * 128, :])

        # ---- qkv projection (phase-scoped SBUF pool) ------------------------
        with tc.tile_pool(name="qkvp", bufs=1) as qp:
            wq = qp.tile([128, 2, 4, 128], f32r)
            wk = qp.tile([128, 2, 4, 128], f32r)
            wv = qp.tile([128, 4, 256], f32r)
            for p in range(2):
                for kc in range(4):
                    nc.sync.dma_start(wq[:, p, kc, :],
                                      wq_d[p, kc * 128:(kc + 1) * 128, :])
                    nc.sync.dma_start(wk[:, p, kc, :],
                                      wk_d[p, kc * 128:(kc + 1) * 128, :])
            for kc in range(4):
                nc.sync.dma_start(wv[:, kc, :], wv_d[kc * 128:(kc + 1) * 128, :])
            xT = qp.tile([128, 4, T], f32r)
            # column-major sub-chunks so the first matmul group's inputs land
            # quickly instead of after the whole 4 MB of x
            for rc4 in range(4):
                for kc in range(4):
                    nc.sync.dma_start(
                        xT[:, kc, rc4 * 512:(rc4 + 1) * 512],
                        xT_d[kc * 128:(kc + 1) * 128, rc4 * 512:(rc4 + 1) * 512])

            for p in range(2):
                for rc4 in range(4):
                    sl = slice(rc4 * 512, (rc4 + 1) * 512)
                    psq = ps_big.tile([128, 512], f32, tag="scores")
                    psk = ps_big.tile([128, 512], f32, tag="scores")
                    for kc in range(4):
                        nc.tensor.matmul(psq[:], wq[:, p, kc, :], xT[:, kc, sl],
                                         start=kc == 0, stop=kc == 3)
                        nc.tensor.matmul(psk[:], wk[:, p, kc, :], xT[:, kc, sl],
                                         start=kc == 0, stop=kc == 3)
                    nc.vector.tensor_copy(qTa[:, p, sl], psq[0:64, :])
                    nc.vector.tensor_copy(kTa[:, p, sl], psk[0:64, :])
                    shq = work.tile([128, 512], f32r, tag="sumscr")
                    nc.vector.tensor_copy(shq[64:128, :], psq[64:128, :])
                    nc.sync.dma_start(qTb[:, p, sl], shq[64:128, :])
                    shk = work.tile([128, 512], f32r, tag="sumscr")
                    nc.vector.tensor_copy(shk[64:128, :], psk[64:128, :])
                    nc.sync.dma_start(kTb[:, p, sl], shk[64:128, :])

            for rc in range(16):
                psv = ps_big.tile([128, 4, 64], f32, tag="scores")
                for kc in range(4):
                    nc.tensor.matmul(psv[:], xT[:, kc, rc * 128:(rc + 1) * 128],
                                     wv[:, kc, :], start=kc == 0, stop=kc == 3)
                nc.vector.tensor_scalar_mul(vsb[:, rc, :, 0:64], psv[:],
                                            mk[:, rc:rc + 1])
                nc.vector.tensor_scalar_mul(vsb[:, rc, :, 64:65], ones41[:],
                                            mk[:, rc:rc + 1])

        # ---- attention (both pairs interleaved per key block) ---------------
        with tc.tile_pool(name="exp", bufs=3) as exp_pool:
            for qc in range(NQC):
                nkb = 2 * (qc + 1)
                qsl = slice(qc * QC, (qc + 1) * QC)
                oo = {}
                for p in range(2):
                    oAp = ps_o.tile([128, 256], f32, tag=f"oA{p}")
                    oBp = ps_o.tile([128, 256], f32, tag=f"oB{p}")
                    oo[p] = (oAp, oBp)
                avq = []
                for kb in range(nkb):
                    ksl = slice(kb * KB, (kb + 1) * KB)
                    sps = ps_big.tile([128, 1024], f32, tag="scores")
                    for p in range(2):
                        nc.tensor.matmul(sps[:, p * 512:p * 512 + 256],
                                         kTa[:, p, ksl], qTa[:, p, qsl],
                                         start=True, stop=True)
                        nc.tensor.matmul(sps[:, p * 512 + 256:p * 512 + 512],
                                         kTb[:, p, ksl], qTb[:, p, qsl],
                                         start=True, stop=True)
                    if kb >= nkb - 2:
                        nc.vector.tensor_add(sps[:], sps[:],
                                             dm[:, kb - (nkb - 2), :])
                    ex = exp_pool.tile([128, 1024], f32r, tag="exp")
                    nc.scalar.activation(ex[:], sps[:], Exp, scale=SCALE)
                    avq.append((kb, ex))
                    if len(avq) > 1:
                        _em(nc, avq.pop(0), oo, vsb, nkb)
                _em(nc, avq.pop(0), oo, vsb, nkb)

                for p in range(2):
                    oA, oB = oo[p]
                    scr = work.tile([128, 512], f32, tag="sumscr")
                    nc.vector.tensor_copy(scr[64:65, 0:256], oA[64:65, :])
                    nc.vector.tensor_copy(scr[64:65, 256:512], oB[64:65, :])
                    idx = p * 8 + qc
                    nc.sync.dma_start(sums_stage[idx:idx + 1, :], scr[64:65, :])
                    nc.vector.tensor_copy(oUA[:, p, qsl], oA[0:64, :])
                    nc.vector.tensor_copy(oUB[:, p, qsl], oB[0:64, :])

        # ---- normalize ------------------------------------------------------
        nc.vector.reciprocal(recips_f[:], sums_stage[:])
        nc.vector.tensor_copy(recips[:], recips_f[:])
        for p in range(2):
            for qc in range(NQC):
                idx = p * 8 + qc
                qsl = slice(qc * QC, (qc + 1) * QC)
                rec = work.tile([1, 512], f32r, tag="rec")
                nc.sync.dma_start(rec[:], recips[idx:idx + 1, :])
                bcA = ps_big.tile([64, 256], f32, tag="scores")
                nc.tensor.matmul(bcA[:], ones64r[:], rec[0:1, 0:256],
                                 start=True, stop=True)
                nc.vector.tensor_mul(oTn2[0:64, p, qsl], oUA[:, p, qsl], bcA[:])
                bcB = ps_big.tile([64, 256], f32, tag="scores")
                nc.tensor.matmul(bcB[:], ones64r[:], rec[0:1, 256:512],
                                 start=True, stop=True)
                scrB = work.tile([64, 256], f32r, tag="scrB")
                nc.vector.tensor_mul(scrB[:], oUB[:, p, qsl], bcB[:])
                # partition shift 0-63 -> 64-127 (DVE lanes are partition-locked)
                nc.sync.dma_start(oTn2[64:128, p, qsl], scrB[:])

        # ---- output projection ---------------------------------------------
        for rc in range(16):
            rsl = slice(rc * 128, (rc + 1) * 128)
            psy = ps_big.tile([128, 512], f32, tag="scores")
            for p in range(2):
                nc.tensor.matmul(psy[:], oTn2[:, p, rsl], wo[:, p, :],
                                 start=p == 0, stop=p == 1)
            yt = work.tile([128, 512], f32, tag="ysb")
            nc.vector.tensor_copy(yt[:], psy[:])
            nc.sync.dma_start(y_d[rsl, :], yt[:])

    nc.compile()
    return nc


def _em(nc, item, oo, vsb, nkb):
    """Emit the deferred A@V accumulations for one key block (4 heads)."""
    kb, ex = item
    for p in range(2):
        oA, oB = oo[p]
        nc.tensor.matmul(oA[0:65, :], vsb[:, kb, 2 * p, :],
                         ex[:, p * 512:p * 512 + 256],
                         start=kb == 0, stop=kb == nkb - 1)
        nc.tensor.matmul(oB[0:65, :], vsb[:, kb, 2 * p + 1, :],
                         ex[:, p * 512 + 256:p * 512 + 512],
                         start=kb == 0, stop=kb == nkb - 1)


def _diag_masks():
    i = np.arange(QC)[None, :]
    j = np.arange(KB)[:, None]
    m0 = np.where(i >= j, 0.0, NEG).astype(np.float32)        # even diag block
    m1 = np.where(i >= j + KB, 0.0, NEG).astype(np.float32)   # odd diag block
    return np.tile(m0, (1, 4)).copy(), np.tile(m1, (1, 4)).copy()


def _prep_inputs(x, m, w_qkv, w_out):
    """Per-core input maps for SPMD dispatch."""
    dm0, dm1 = _diag_masks()
    dm4 = np.stack([dm0, dm1]).astype(np.float32)
    wq_full = w_qkv[:, 0:D]
    wk_full = w_qkv[:, D:2 * D]
    wv_full = w_qkv[:, 2 * D:3 * D]
    in_maps = []
    for c in range(8):
        b, q = c // 2, c % 2
        hsl = slice(4 * q * DH, (4 * q + 4) * DH)
        wq2 = np.stack([
            np.concatenate([wq_full[:, (4 * q + 2 * p) * DH:(4 * q + 2 * p + 1) * DH],
                            wq_full[:, (4 * q + 2 * p + 1) * DH:(4 * q + 2 * p + 2) * DH]],
                           axis=1)
            for p in range(2)])
        wk2 = np.stack([
            np.concatenate([wk_full[:, (4 * q + 2 * p) * DH:(4 * q + 2 * p + 1) * DH],
                            wk_full[:, (4 * q + 2 * p + 1) * DH:(4 * q + 2 * p + 2) * DH]],
                           axis=1)
            for p in range(2)])
        in_maps.append({
            "xT": np.ascontiguousarray(x[b].T).astype(np.float32),
            "wq2": np.ascontiguousarray(wq2).astype(np.float32),
            "wk2": np.ascontiguousarray(wk2).astype(np.float32),
            "wv4": np.ascontiguousarray(wv_full[:, hsl]).astype(np.float32),
            "wo4": np.ascontiguousarray(w_out[hsl, :]).astype(np.float32),
            "dm4": dm4,
            "mkey": np.ascontiguousarray((m[b] != 0).astype(np.float32)[:, None]),
        })
    return in_maps


def _execute(inputs, trace=False):
    from concourse.bass_utils import run_bass_kernel_spmd

    if "nc" not in _CACHE:
        _CACHE["nc"] = _build_program()
    nc = _CACHE["nc"]

    x = np.asarray(inputs["x"], np.float32)
    m = np.asarray(inputs["m"], np.float32)
    w_qkv = np.asarray(inputs["w_qkv"], np.float32)
    w_out = np.asarray(inputs["w_out"], np.float32)
    b_out = np.asarray(inputs["b_out"], np.float32)

    in_maps = _prep_inputs(x, m, w_qkv, w_out)
    res = run_bass_kernel_spmd(nc, in_maps, core_ids=list(range(8)), trace=trace)

    y = np.empty((B, T, D), np.float32)
    for b in range(B):
        y[b] = res.results[2 * b]["y"] + res.results[2 * b + 1]["y"]
    y += b_out[None, None, :]
    y *= m[..., None]
    return y, res


def kernel(**inputs) -> np.ndarray:
    y, _ = _execute(inputs, trace=False)
    return y
